# revision 30
# baseline (speedup 1.0000x reference)
"""Multi-head attention (B=2, S=2048, D=1024, H=16) on 8 trn2 NeuronCores.

Sharding: core c = (b, g) with b = c // 4 (data parallel over batch) and
g = c % 4 (tensor parallel over heads, 4 heads per core).  Each core
computes q/k/v projections for its 4 heads, attention, and a partial
output projection (row-parallel Wo); the host sums the 4 partials per
batch and adds bo.

All activations are laid out so that no on-chip transpose is needed:
the host passes Q/K/V pre-transposed ([D, S]) and weights pre-sliced/
transposed.  q,k are computed transposed ([dk, s]); v natural ([s, dk]).
scores_T = k @ q.T is computed with K=64 row-packed matmul pairs (two
heads concurrently in the 128x128 PE array).  Softmax skips the max
subtraction (scores are O(1) for these inputs) and gets its denominators
for free from a ones-column appended to v in the P@V matmul.
"""

import contextlib
import sys

import numpy as np

for _p in ("/opt/trn_rl_repo", "/root/.axon_site/_ro/trn_rl_repo"):
    if _p not in sys.path:
        sys.path.insert(0, _p)

B, S, D = 2, 2048, 1024
H, DK = 16, 64
HPC = 4  # heads per core
HD = HPC * DK  # 256 head-dims per core
NCORES = 8
SCALE = 1.0 / 8.0  # 1/sqrt(DK)

_CACHE = {}


def _build_nc():
    from concourse import bacc
    import concourse.mybir as mybir
    import concourse.tile as tile

    F32 = mybir.dt.float32
    BF16 = mybir.dt.bfloat16
    Exp = mybir.ActivationFunctionType.Exp

    nc = bacc.Bacc(None)

    qt_d = nc.dram_tensor("qt", [D, S], BF16, kind="ExternalInput")
    kt_d = nc.dram_tensor("kt", [D, S], BF16, kind="ExternalInput")
    vt_d = nc.dram_tensor("vt", [D, S], BF16, kind="ExternalInput")
    wqt_d = nc.dram_tensor("wqt", [D, HD], BF16, kind="ExternalInput")
    wkt_d = nc.dram_tensor("wkt", [D, HD], BF16, kind="ExternalInput")
    wvt_d = nc.dram_tensor("wvt", [D, HD], BF16, kind="ExternalInput")
    wot_d = nc.dram_tensor("wot", [HD, D], BF16, kind="ExternalInput")
    bq_d = nc.dram_tensor("bq", [HD], F32, kind="ExternalInput")
    bk_d = nc.dram_tensor("bk", [HD], F32, kind="ExternalInput")
    bv_d = nc.dram_tensor("bv", [HD], F32, kind="ExternalInput")
    out_d = nc.dram_tensor("out", [S, D], F32, kind="ExternalOutput")

    KT = D // 128  # 8 contraction tiles for the projections
    NS = S // 512  # 4 sq tiles
    NB = S // 128  # 16 sk blocks / sq row-blocks

    with tile.TileContext(nc) as tc, contextlib.ExitStack() as ctx:
        consts = ctx.enter_context(tc.tile_pool(name="consts", bufs=1))
        wpool = ctx.enter_context(tc.tile_pool(name="wpool", bufs=2))
        xt = ctx.enter_context(tc.tile_pool(name="xt", bufs=14))
        persist = ctx.enter_context(tc.tile_pool(name="persist", bufs=1))
        probsp = ctx.enter_context(tc.tile_pool(name="probsp", bufs=4))
        smallp = ctx.enter_context(tc.tile_pool(name="smallp", bufs=2))
        outp = ctx.enter_context(tc.tile_pool(name="outp", bufs=3))
        psum = ctx.enter_context(tc.tile_pool(name="psum", bufs=2, space="PSUM"))

        # ---- constants ----
        bq_sb = consts.tile([128, 2], F32)  # col m = bq[128m : 128(m+1)]
        nc.sync.dma_start(out=bq_sb[:], in_=bq_d[:].rearrange("(m p) -> p m", p=128))
        bk_sb = consts.tile([128, 2], F32)
        nc.sync.dma_start(out=bk_sb[:], in_=bk_d[:].rearrange("(m p) -> p m", p=128))
        bv_sb = consts.tile([64, 4], F32)  # col h = bv[64h : 64(h+1)]
        nc.sync.dma_start(out=bv_sb[:], in_=bv_d[:].rearrange("(h q) -> q h", q=64))

        wo_sb = consts.tile([128, 2, D], BF16)  # [p][pair][dmodel]

        # ---- persistent activations ----
        qT = [persist.tile([128, S], BF16, name=f"qT{m}") for m in range(2)]
        kT = [persist.tile([128, S], BF16, name=f"kT{m}") for m in range(2)]
        # v with a ones column appended per head: [s-block][128, head, 65]
        vplus = [persist.tile([128, HPC, DK + 1], BF16, name=f"vp{i}") for i in range(NB)]
        ones_sb = consts.tile([128, HPC], F32)
        nc.vector.memset(ones_sb[:], 1.0)
        for i in range(NB):
            nc.vector.tensor_copy(
                vplus[i][:, :, DK : DK + 1], ones_sb[:].rearrange("p (h o) -> p h o", o=1)
            )
        attnT = [persist.tile([128, S], BF16, name=f"attnT{p}") for p in range(2)]

        # ---- projections ----
        def proj_group(w_sb, bias_sb, dst, x_tiles, m, s):
            # one accumulation group: dst[m][:, 512s:512(s+1)] = W @ X.T + b
            ps = psum.tile([128, 512], F32, tag="big", bufs=2)
            for k in range(KT):
                nc.tensor.matmul(
                    ps[:],
                    w_sb[:, k, m * 128 : (m + 1) * 128],
                    x_tiles[k][:, s * 512 : (s + 1) * 512],
                    start=(k == 0),
                    stop=(k == KT - 1),
                )
            nc.vector.tensor_scalar_add(
                dst[m][:, s * 512 : (s + 1) * 512], ps[:], bias_sb[:, m : m + 1]
            )

        def vproj_group(sb):
            ps = psum.tile([128, HD], F32, tag="acc", bufs=2)
            for k in range(KT):
                nc.tensor.matmul(
                    ps[:],
                    v_tiles[k][:, sb * 128 : (sb + 1) * 128],
                    wv_sb[:, k, :],
                    start=(k == 0),
                    stop=(k == KT - 1),
                )
            nc.vector.tensor_copy(
                vplus[sb][:, :, 0:DK], ps[:].rearrange("p (h d) -> p h d", h=HPC)
            )

        def load_x(x_dram):
            tiles = []
            for k in range(KT):
                t = xt.tile([128, S], BF16, tag="xt")
                nc.sync.dma_start(
                    out=t[:, 0 : S // 2],
                    in_=x_dram[k * 128 : (k + 1) * 128, 0 : S // 2],
                )
                nc.sync.dma_start(
                    out=t[:, S // 2 : S],
                    in_=x_dram[k * 128 : (k + 1) * 128, S // 2 : S],
                )
                tiles.append(t)
            return tiles

        # weights first (small, they gate the first matmuls), then the bulk
        # activation loads
        wk_sb = wpool.tile([128, KT, HD], BF16, tag="wproj", bufs=3)
        nc.sync.dma_start(
            out=wk_sb[:], in_=wkt_d[:].rearrange("(kt p) m -> p kt m", p=128)
        )
        wq_sb = wpool.tile([128, KT, HD], BF16, tag="wproj", bufs=3)
        nc.sync.dma_start(
            out=wq_sb[:], in_=wqt_d[:].rearrange("(kt p) m -> p kt m", p=128)
        )
        wv_sb = wpool.tile([128, KT, HD], BF16, tag="wproj", bufs=3)
        nc.sync.dma_start(
            out=wv_sb[:], in_=wvt_d[:].rearrange("(kt p) m -> p kt m", p=128)
        )
        k_tiles = load_x(kt_d)
        q_tiles = load_x(qt_d)
        v_tiles = load_x(vt_d)
        nc.sync.dma_start(
            out=wo_sb[:], in_=wot_d[:].rearrange("(m p) n -> p m n", p=128)
        )

        def scores_exp(t, p, sb, probs):
            tsl = slice(t * 512, (t + 1) * 512)
            ps_sc = psum.tile([128, 1024], F32, tag="big", bufs=2)
            # scores_T = k_h @ q_h.T for both heads of the pair,
            # row-packed into the two 64-row halves of the PE array
            for j in range(2):  # head j of pair: partitions 64j..64j+64
                hsl = slice(64 * j, 64 * (j + 1))
                nc.tensor.matmul(
                    ps_sc[:, j * 512 : (j + 1) * 512],
                    kT[p][hsl, sb * 128 : (sb + 1) * 128],
                    qT[p][hsl, tsl],
                    start=True,
                    stop=True,
                    tile_position=(64 * j, 0),
                )
            nc.scalar.activation(probs[:], ps_sc[:], Exp, scale=SCALE)

        probs_ctr = [0]

        def new_probs():
            probs_ctr[0] += 1
            return probsp.tile(
                [128, 1024], BF16, tag="probs", bufs=NB + 3,
                name=f"probs{probs_ctr[0]}",
            )

        def pv(ps_at, p, sb, probs):
            for j in range(2):
                nc.tensor.matmul(
                    ps_at[:, j * 512 : (j + 1) * 512],
                    vplus[sb][:, 2 * p + j, :],
                    probs[:, j * 512 : (j + 1) * 512],
                    start=(sb == 0),
                    stop=(sb == NB - 1),
                )

        def normalize(t, p, ps_at):
            # attn = attn_unnorm / sumexp, + bv
            tsl = slice(t * 512, (t + 1) * 512)
            sums = smallp.tile([1, 1024], F32, tag="sums")
            nc.vector.tensor_copy(sums[:], ps_at[DK : DK + 1, :])
            recip = smallp.tile([1, 1024], F32, tag="recip")
            nc.vector.reciprocal_approx_fast(recip[:], sums[:])
            rec_b = smallp.tile([64, 1024], F32, tag="rec_b", bufs=1)
            nc.gpsimd.partition_broadcast(rec_b[:], recip[0:1, :])
            tmp = smallp.tile([64, 1024], BF16, tag="tmpn")
            nc.vector.tensor_mul(tmp[:], ps_at[0:DK, :], rec_b[:])
            for j in range(2):
                jsl = slice(j * 512, (j + 1) * 512)
                nc.vector.tensor_scalar_add(
                    tmp[:, jsl], tmp[:, jsl], bv_sb[:, 2 * p + j : 2 * p + j + 1]
                )
            # head j=0 -> partitions 0:64 of attnT[p]; j=1 -> 64:128
            nc.vector.tensor_copy(attnT[p][0:64, tsl], tmp[:, 0:512])
            nc.sync.dma_start(out=attnT[p][64:128, tsl], in_=tmp[:, 512:1024])

        def out_proj(t):
            # output projection for the 4 row-blocks of sq tile t
            for sb in range(4 * t, 4 * t + 4):
                ps_o = psum.tile([128, 1024], F32, tag="big", bufs=2)
                for n in range(2):
                    for p in range(2):
                        nc.tensor.matmul(
                            ps_o[:, n * 512 : (n + 1) * 512],
                            attnT[p][:, sb * 128 : (sb + 1) * 128],
                            wo_sb[:, p, n * 512 : (n + 1) * 512],
                            start=(p == 0),
                            stop=(p == 1),
                        )
                o_sb = outp.tile([128, 1024], F32, tag="osb")
                # ScalarE copy: keeps big-psum slot recycling off the DVE queue
                nc.scalar.copy(o_sb[:], ps_o[:])
                nc.sync.dma_start(
                    out=out_d[sb * 128 : (sb + 1) * 128, :], in_=o_sb[:]
                )

        # ---- projections: k then q (both m-blocks); the exp pre-stage
        # sits after so the ACT engine starts before the v-projection ----
        for m in range(2):
            for s in range(NS):
                proj_group(wk_sb, bk_sb, kT, k_tiles, m, s)
        for m in range(2):
            for s in range(NS):
                proj_group(wq_sb, bq_sb, qT, q_tiles, m, s)

        # bank 16 exp tiles for (t=0, p=0)
        pre0 = []
        for sb in range(NB):
            probs = new_probs()
            scores_exp(0, 0, sb, probs)
            pre0.append(probs)

        for sb in range(NB):
            vproj_group(sb)

        # ---- t=0: consume p0 probs while producing p1 probs so ACT never
        # idles between the two pair phases
        ps_at00 = psum.tile([DK + 1, 1024], F32, tag="acc", bufs=2)
        pre1 = []
        for sb in range(NB):
            pv(ps_at00, 0, sb, pre0[sb])
            probs = new_probs()
            scores_exp(0, 1, sb, probs)
            pre1.append(probs)
        normalize(0, 0, ps_at00)
        ps_at01 = psum.tile([DK + 1, 1024], F32, tag="acc", bufs=2)
        for sb in range(NB):
            pv(ps_at01, 1, sb, pre1[sb])
        normalize(0, 1, ps_at01)

        # ---- steady state ----
        for t in range(1, NS):
            for p in range(2):
                ps_at = psum.tile([DK + 1, 1024], F32, tag="acc", bufs=2)
                for sb in range(NB):
                    probs = new_probs()
                    scores_exp(t, p, sb, probs)
                    pv(ps_at, p, sb, probs)
                normalize(t, p, ps_at)
                if p == 0:
                    # previous tile's output projection: by now its attnT
                    # inputs have been ready for a full pair-phase
                    out_proj(t - 1)

        out_proj(NS - 1)

    nc.finalize()
    return nc


def kernel(Q, K, V, Wq, bq, Wk, bk, Wv, bv, Wo, bo):
    from concourse.bass_utils import run_bass_kernel_spmd

    Q, K, V = (np.asarray(a, dtype=np.float32) for a in (Q, K, V))
    Wq, bq, Wk, bk = (np.asarray(a, dtype=np.float32) for a in (Wq, bq, Wk, bk))
    Wv, bv, Wo, bo = (np.asarray(a, dtype=np.float32) for a in (Wv, bv, Wo, bo))

    if "nc" not in _CACHE:
        _CACHE["nc"] = _build_nc()
    nc = _CACHE["nc"]

    import ml_dtypes

    bf16 = ml_dtypes.bfloat16
    qts = [np.ascontiguousarray(Q[b].T).astype(bf16) for b in range(B)]
    kts = [np.ascontiguousarray(K[b].T).astype(bf16) for b in range(B)]
    vts = [np.ascontiguousarray(V[b].T).astype(bf16) for b in range(B)]
    in_maps = []
    for c in range(NCORES):
        b, g = divmod(c, 4)
        sl = slice(g * HD, (g + 1) * HD)
        in_maps.append(
            {
                "qt": qts[b],
                "kt": kts[b],
                "vt": vts[b],
                "wqt": np.ascontiguousarray(Wq[sl, :].T).astype(bf16),
                "wkt": np.ascontiguousarray(Wk[sl, :].T).astype(bf16),
                "wvt": np.ascontiguousarray(Wv[sl, :].T).astype(bf16),
                "wot": np.ascontiguousarray(Wo[:, sl].T).astype(bf16),
                "bq": np.ascontiguousarray(bq[sl]),
                "bk": np.ascontiguousarray(bk[sl]),
                "bv": np.ascontiguousarray(bv[sl]),
            }
        )

    res = run_bass_kernel_spmd(nc, in_maps, core_ids=list(range(NCORES)))

    out = np.zeros((B, S, D), dtype=np.float32)
    for c in range(NCORES):
        out[c // 4] += res.results[c]["out"]
    out += bo
    return out


# revision 31
# speedup vs baseline: 1.0194x; 1.0194x over previous
"""Multi-head attention (B=2, S=2048, D=1024, H=16) on 8 trn2 NeuronCores.

Sharding: core c = (b, g) with b = c // 4 (data parallel over batch) and
g = c % 4 (tensor parallel over heads, 4 heads per core).  Each core
computes q/k/v projections for its 4 heads, attention, and a partial
output projection (row-parallel Wo); the host sums the 4 partials per
batch and adds bo.

All activations are laid out so that no on-chip transpose is needed:
the host passes Q/K/V pre-transposed ([D, S]) and weights pre-sliced/
transposed.  q,k are computed transposed ([dk, s]); v natural ([s, dk]).
scores_T = k @ q.T is computed with K=64 row-packed matmul pairs (two
heads concurrently in the 128x128 PE array).  Softmax skips the max
subtraction (scores are O(1) for these inputs) and gets its denominators
for free from a ones-column appended to v in the P@V matmul.
"""

import contextlib
import sys

import numpy as np

for _p in ("/opt/trn_rl_repo", "/root/.axon_site/_ro/trn_rl_repo"):
    if _p not in sys.path:
        sys.path.insert(0, _p)

B, S, D = 2, 2048, 1024
H, DK = 16, 64
HPC = 4  # heads per core
HD = HPC * DK  # 256 head-dims per core
NCORES = 8
SCALE = 1.0 / 8.0  # 1/sqrt(DK)

_CACHE = {}


def _build_nc():
    from concourse import bacc
    import concourse.mybir as mybir
    import concourse.tile as tile

    F32 = mybir.dt.float32
    BF16 = mybir.dt.bfloat16
    Exp = mybir.ActivationFunctionType.Exp

    nc = bacc.Bacc(None)

    qt_d = nc.dram_tensor("qt", [D, S], BF16, kind="ExternalInput")
    kt_d = nc.dram_tensor("kt", [D, S], BF16, kind="ExternalInput")
    vt_d = nc.dram_tensor("vt", [D, S], BF16, kind="ExternalInput")
    wqt_d = nc.dram_tensor("wqt", [D, HD], BF16, kind="ExternalInput")
    wkt_d = nc.dram_tensor("wkt", [D, HD], BF16, kind="ExternalInput")
    wvt_d = nc.dram_tensor("wvt", [D, HD], BF16, kind="ExternalInput")
    wot_d = nc.dram_tensor("wot", [HD, D], BF16, kind="ExternalInput")
    bq_d = nc.dram_tensor("bq", [HD], F32, kind="ExternalInput")
    bk_d = nc.dram_tensor("bk", [HD], F32, kind="ExternalInput")
    bv_d = nc.dram_tensor("bv", [HD], F32, kind="ExternalInput")
    out_d = nc.dram_tensor("out", [S, D], F32, kind="ExternalOutput")

    KT = D // 128  # 8 contraction tiles for the projections
    NS = S // 512  # 4 sq tiles
    NB = S // 128  # 16 sk blocks / sq row-blocks

    with tile.TileContext(nc) as tc, contextlib.ExitStack() as ctx:
        consts = ctx.enter_context(tc.tile_pool(name="consts", bufs=1))
        wpool = ctx.enter_context(tc.tile_pool(name="wpool", bufs=2))
        xt = ctx.enter_context(tc.tile_pool(name="xt", bufs=16))
        persist = ctx.enter_context(tc.tile_pool(name="persist", bufs=1))
        probsp = ctx.enter_context(tc.tile_pool(name="probsp", bufs=4))
        smallp = ctx.enter_context(tc.tile_pool(name="smallp", bufs=2))
        outp = ctx.enter_context(tc.tile_pool(name="outp", bufs=3))
        psum = ctx.enter_context(tc.tile_pool(name="psum", bufs=2, space="PSUM"))

        # ---- constants ----
        bq_sb = consts.tile([128, 2], F32)  # col m = bq[128m : 128(m+1)]
        nc.sync.dma_start(out=bq_sb[:], in_=bq_d[:].rearrange("(m p) -> p m", p=128))
        bk_sb = consts.tile([128, 2], F32)
        nc.sync.dma_start(out=bk_sb[:], in_=bk_d[:].rearrange("(m p) -> p m", p=128))
        bv_sb = consts.tile([64, 4], F32)  # col h = bv[64h : 64(h+1)]
        nc.sync.dma_start(out=bv_sb[:], in_=bv_d[:].rearrange("(h q) -> q h", q=64))

        wo_sb = consts.tile([128, 2, D], BF16)  # [p][pair][dmodel]

        # ---- persistent activations ----
        qT = [persist.tile([128, S], BF16, name=f"qT{m}") for m in range(2)]
        kT = [persist.tile([128, S], BF16, name=f"kT{m}") for m in range(2)]
        # v with a ones column appended per head: [s-block][128, head, 65]
        vplus = [persist.tile([128, HPC, DK + 1], BF16, name=f"vp{i}") for i in range(NB)]
        ones_sb = consts.tile([128, HPC], F32)
        nc.vector.memset(ones_sb[:], 1.0)
        for i in range(NB):
            nc.vector.tensor_copy(
                vplus[i][:, :, DK : DK + 1], ones_sb[:].rearrange("p (h o) -> p h o", o=1)
            )
        attnT = [persist.tile([128, S], BF16, name=f"attnT{p}") for p in range(2)]

        # ---- projections ----
        def proj_group(w_sb, bias_sb, dst, x_tiles, m, s):
            # one accumulation group: dst[m][:, 512s:512(s+1)] = W @ X.T + b
            ps = psum.tile([128, 512], F32, tag="big", bufs=2)
            for k in range(KT):
                nc.tensor.matmul(
                    ps[:],
                    w_sb[:, k, m * 128 : (m + 1) * 128],
                    x_tiles[k][:, s * 512 : (s + 1) * 512],
                    start=(k == 0),
                    stop=(k == KT - 1),
                )
            nc.vector.tensor_scalar_add(
                dst[m][:, s * 512 : (s + 1) * 512], ps[:], bias_sb[:, m : m + 1]
            )

        def vproj_group(sb):
            ps = psum.tile([128, HD], F32, tag="acc", bufs=2)
            for k in range(KT):
                nc.tensor.matmul(
                    ps[:],
                    v_tiles[k][:, sb * 128 : (sb + 1) * 128],
                    wv_sb[:, k, :],
                    start=(k == 0),
                    stop=(k == KT - 1),
                )
            nc.vector.tensor_copy(
                vplus[sb][:, :, 0:DK], ps[:].rearrange("p (h d) -> p h d", h=HPC)
            )

        def load_x(x_dram):
            tiles = []
            for k in range(KT):
                t = xt.tile([128, S], BF16, tag="xt")
                nc.sync.dma_start(
                    out=t[:, 0 : S // 2],
                    in_=x_dram[k * 128 : (k + 1) * 128, 0 : S // 2],
                )
                nc.sync.dma_start(
                    out=t[:, S // 2 : S],
                    in_=x_dram[k * 128 : (k + 1) * 128, S // 2 : S],
                )
                tiles.append(t)
            return tiles

        # weights first (small, they gate the first matmuls), then the bulk
        # activation loads
        wk_sb = wpool.tile([128, KT, HD], BF16, tag="wproj", bufs=3)
        nc.sync.dma_start(
            out=wk_sb[:], in_=wkt_d[:].rearrange("(kt p) m -> p kt m", p=128)
        )
        wq_sb = wpool.tile([128, KT, HD], BF16, tag="wproj", bufs=3)
        nc.sync.dma_start(
            out=wq_sb[:], in_=wqt_d[:].rearrange("(kt p) m -> p kt m", p=128)
        )
        wv_sb = wpool.tile([128, KT, HD], BF16, tag="wproj", bufs=3)
        nc.sync.dma_start(
            out=wv_sb[:], in_=wvt_d[:].rearrange("(kt p) m -> p kt m", p=128)
        )
        k_tiles = load_x(kt_d)
        q_tiles = load_x(qt_d)
        v_tiles = load_x(vt_d)
        nc.sync.dma_start(
            out=wo_sb[:], in_=wot_d[:].rearrange("(m p) n -> p m n", p=128)
        )

        def scores_exp(t, p, sb, probs):
            tsl = slice(t * 512, (t + 1) * 512)
            ps_sc = psum.tile([128, 1024], F32, tag="big", bufs=2)
            # scores_T = k_h @ q_h.T for both heads of the pair,
            # row-packed into the two 64-row halves of the PE array
            for j in range(2):  # head j of pair: partitions 64j..64j+64
                hsl = slice(64 * j, 64 * (j + 1))
                nc.tensor.matmul(
                    ps_sc[:, j * 512 : (j + 1) * 512],
                    kT[p][hsl, sb * 128 : (sb + 1) * 128],
                    qT[p][hsl, tsl],
                    start=True,
                    stop=True,
                    tile_position=(64 * j, 0),
                )
            nc.scalar.activation(probs[:], ps_sc[:], Exp, scale=SCALE)

        probs_ctr = [0]

        def new_probs():
            probs_ctr[0] += 1
            return probsp.tile(
                [128, 1024], BF16, tag="probs", bufs=NB + 3,
                name=f"probs{probs_ctr[0]}",
            )

        def pv(ps_at, p, sb, probs):
            for j in range(2):
                nc.tensor.matmul(
                    ps_at[:, j * 512 : (j + 1) * 512],
                    vplus[sb][:, 2 * p + j, :],
                    probs[:, j * 512 : (j + 1) * 512],
                    start=(sb == 0),
                    stop=(sb == NB - 1),
                )

        def normalize(t, p, ps_at):
            # attn = attn_unnorm / sumexp, + bv
            tsl = slice(t * 512, (t + 1) * 512)
            sums = smallp.tile([1, 1024], F32, tag="sums")
            nc.vector.tensor_copy(sums[:], ps_at[DK : DK + 1, :])
            recip = smallp.tile([1, 1024], F32, tag="recip")
            nc.vector.reciprocal_approx_fast(recip[:], sums[:])
            rec_b = smallp.tile([64, 1024], F32, tag="rec_b", bufs=1)
            nc.gpsimd.partition_broadcast(rec_b[:], recip[0:1, :])
            tmp = smallp.tile([64, 1024], BF16, tag="tmpn")
            nc.vector.tensor_mul(tmp[:], ps_at[0:DK, :], rec_b[:])
            for j in range(2):
                jsl = slice(j * 512, (j + 1) * 512)
                nc.vector.tensor_scalar_add(
                    tmp[:, jsl], tmp[:, jsl], bv_sb[:, 2 * p + j : 2 * p + j + 1]
                )
            # head j=0 -> partitions 0:64 of attnT[p]; j=1 -> 64:128
            nc.vector.tensor_copy(attnT[p][0:64, tsl], tmp[:, 0:512])
            nc.sync.dma_start(out=attnT[p][64:128, tsl], in_=tmp[:, 512:1024])

        def out_proj(t):
            # output projection for the 4 row-blocks of sq tile t
            for sb in range(4 * t, 4 * t + 4):
                ps_o = psum.tile([128, 1024], F32, tag="big", bufs=2)
                for n in range(2):
                    for p in range(2):
                        nc.tensor.matmul(
                            ps_o[:, n * 512 : (n + 1) * 512],
                            attnT[p][:, sb * 128 : (sb + 1) * 128],
                            wo_sb[:, p, n * 512 : (n + 1) * 512],
                            start=(p == 0),
                            stop=(p == 1),
                        )
                o_sb = outp.tile([128, 1024], F32, tag="osb")
                # ScalarE copy: keeps big-psum slot recycling off the DVE queue
                nc.scalar.copy(o_sb[:], ps_o[:])
                nc.sync.dma_start(
                    out=out_d[sb * 128 : (sb + 1) * 128, :], in_=o_sb[:]
                )

        # ---- projections: k then q (both m-blocks); the exp pre-stage
        # sits after so the ACT engine starts before the v-projection ----
        for m in range(2):
            for s in range(NS):
                proj_group(wk_sb, bk_sb, kT, k_tiles, m, s)
        for m in range(2):
            for s in range(NS):
                proj_group(wq_sb, bq_sb, qT, q_tiles, m, s)

        # bank 16 exp tiles for (t=0, p=0)
        pre0 = []
        for sb in range(NB):
            probs = new_probs()
            scores_exp(0, 0, sb, probs)
            pre0.append(probs)

        for sb in range(NB):
            vproj_group(sb)

        # ---- t=0: consume p0 probs while producing p1 probs so ACT never
        # idles between the two pair phases
        ps_at00 = psum.tile([DK + 1, 1024], F32, tag="acc", bufs=2)
        pre1 = []
        for sb in range(NB):
            pv(ps_at00, 0, sb, pre0[sb])
            probs = new_probs()
            scores_exp(0, 1, sb, probs)
            pre1.append(probs)
        normalize(0, 0, ps_at00)
        ps_at01 = psum.tile([DK + 1, 1024], F32, tag="acc", bufs=2)
        for sb in range(NB):
            pv(ps_at01, 1, sb, pre1[sb])
        normalize(0, 1, ps_at01)

        # ---- steady state ----
        for t in range(1, NS):
            for p in range(2):
                ps_at = psum.tile([DK + 1, 1024], F32, tag="acc", bufs=2)
                for sb in range(NB):
                    probs = new_probs()
                    scores_exp(t, p, sb, probs)
                    pv(ps_at, p, sb, probs)
                normalize(t, p, ps_at)
                if p == 0:
                    # previous tile's output projection: by now its attnT
                    # inputs have been ready for a full pair-phase
                    out_proj(t - 1)

        out_proj(NS - 1)

    nc.finalize()
    return nc


def kernel(Q, K, V, Wq, bq, Wk, bk, Wv, bv, Wo, bo):
    from concourse.bass_utils import run_bass_kernel_spmd

    Q, K, V = (np.asarray(a, dtype=np.float32) for a in (Q, K, V))
    Wq, bq, Wk, bk = (np.asarray(a, dtype=np.float32) for a in (Wq, bq, Wk, bk))
    Wv, bv, Wo, bo = (np.asarray(a, dtype=np.float32) for a in (Wv, bv, Wo, bo))

    if "nc" not in _CACHE:
        _CACHE["nc"] = _build_nc()
    nc = _CACHE["nc"]

    import ml_dtypes

    bf16 = ml_dtypes.bfloat16
    qts = [np.ascontiguousarray(Q[b].T).astype(bf16) for b in range(B)]
    kts = [np.ascontiguousarray(K[b].T).astype(bf16) for b in range(B)]
    vts = [np.ascontiguousarray(V[b].T).astype(bf16) for b in range(B)]
    in_maps = []
    for c in range(NCORES):
        b, g = divmod(c, 4)
        sl = slice(g * HD, (g + 1) * HD)
        in_maps.append(
            {
                "qt": qts[b],
                "kt": kts[b],
                "vt": vts[b],
                "wqt": np.ascontiguousarray(Wq[sl, :].T).astype(bf16),
                "wkt": np.ascontiguousarray(Wk[sl, :].T).astype(bf16),
                "wvt": np.ascontiguousarray(Wv[sl, :].T).astype(bf16),
                "wot": np.ascontiguousarray(Wo[:, sl].T).astype(bf16),
                "bq": np.ascontiguousarray(bq[sl]),
                "bk": np.ascontiguousarray(bk[sl]),
                "bv": np.ascontiguousarray(bv[sl]),
            }
        )

    res = run_bass_kernel_spmd(nc, in_maps, core_ids=list(range(NCORES)))

    out = np.zeros((B, S, D), dtype=np.float32)
    for c in range(NCORES):
        out[c // 4] += res.results[c]["out"]
    out += bo
    return out
